# revision 1
# baseline (speedup 1.0000x reference)
"""Trainium2 Bass kernel for nn_Decomp_Forecast (HiPPO-LegS decomposition forecaster).

Math: the reference runs a 720-step linear scan c_t = c_{t-1} @ A^T + f_t * B
and only uses the final state, so the whole model collapses (exactly, by
associativity) to two chained matmuls around the instance-norm statistics:

    G[t]   = B^T (A^T)^(T-1-t)            (host-folded, float64)  [720, 64]
    P      = eval_matrix @ W_mlp                                   [720, 64]
    v      = eval_matrix @ b_mlp                                   [720]
    q      = P @ sum_t G[t]                                        [720]

    U      = x_row @ G      (x_row = raw x_enc[b, :, e], no normalization!)
    mu     = mean_t(x_row);  sd = sqrt(var_t(x_row) + 1e-5)
    out[t', r] = (P @ U)[t'] + mu_r * (1 - q[t']) + sd_r * v[t']

(the affine weight/bias are ones/zeros per the model setup, and the RevIN
scale cancels through the linear path, leaving the rank-2 mu/sd correction,
which is folded into the second matmul as two extra contraction rows.)

Device kernel per core (2 batches of the 16, data-parallel over batch):
  - time dim mapped as t = p*6 + a (p = SBUF partition, a = column block) so
    every DMA moves 7.7KB contiguous runs per partition (descriptor-efficient)
  - all matmul operands live in float32r (TF32-style PE mode, 1 cycle/row for
    even moving dims >= 256 -> channel dim host-padded 321 -> 322)
  phase A: 6 k-tile matmuls [120t x 66] x [120t x 322e] -> psum [66, 322]
           rows 0,1 = sum_t x (two ones cols in W1), rows 2..65 = U^T
           + 6 matmuls of the squared tiles -> psum_s rows 0,1 = sum_t x^2
  phase B: tiny [2, 322] vector ops -> rhs2 row 0 = mu, row 1 = sd
  phase C: 6 matmuls [66 x 120] x [66 x 322] -> out tiles -> one DMA per batch
"""

import numpy as np

BATCH, T, E, N = 16, 720, 321, 64
N_CORES = 8
B_PER_CORE = BATCH // N_CORES   # 2
TT = 120                        # time-tile (partition dim of phase-A matmuls)
NT = T // TT                    # 6
M1 = N + 2                      # 66: two ones columns + G columns
EP = E + 1                      # 322: fp32r matmul moving dim must be even

_PROGRAM = None


def _fold_weights(A, B_vec, eval_matrix, W_mlp, b_mlp):
    """Host-side weight folding in float64.

    Returns W1 [720, 66] (cols: [1, 1, G]) and W2 reordered to [66, 6, 120]
    (rows: [1-q, v, P^T], columns regrouped so block a holds t' = p*6 + a).
    """
    A64 = np.asarray(A, np.float64)
    Bv = np.asarray(B_vec, np.float64)
    G = np.empty((T, N), np.float64)
    r = Bv.copy()                       # r_k = B^T (A^T)^k
    for k in range(T):
        G[T - 1 - k] = r
        r = r @ A64.T
    P_mat = np.asarray(eval_matrix, np.float64) @ np.asarray(W_mlp, np.float64)
    v = np.asarray(eval_matrix, np.float64) @ np.asarray(b_mlp, np.float64)
    q = P_mat @ G.sum(axis=0)
    W1 = np.concatenate([np.ones((T, 2)), G], axis=1).astype(np.float32)
    W2 = np.concatenate(
        [(1.0 - q)[None, :], v[None, :], P_mat.T], axis=0
    ).astype(np.float32)
    W2 = W2.reshape(M1, TT, NT).transpose(0, 2, 1)      # [66, 6, 120]
    return np.ascontiguousarray(W1), np.ascontiguousarray(W2)


def _build_program():
    from contextlib import ExitStack

    import concourse.tile as tile
    from concourse import bacc, mybir

    f32 = mybir.dt.float32
    f32r = mybir.dt.float32r
    nc = bacc.Bacc("TRN2", target_bir_lowering=False, debug=False,
                   num_devices=N_CORES)

    # xs is host-padded to EP columns (zeros) and declared float32r: the DMA
    # feeds the PE directly with no on-chip cast pass.
    xs = nc.dram_tensor("xs", [B_PER_CORE, T, EP], f32r, kind="ExternalInput")
    w1 = nc.dram_tensor("w1", [T, M1], f32r, kind="ExternalInput")
    w2 = nc.dram_tensor("w2", [M1, NT, TT], f32r, kind="ExternalInput")
    out = nc.dram_tensor("out", [B_PER_CORE, T, E], f32, kind="ExternalOutput")

    with tile.TileContext(nc) as tc, ExitStack() as ctx:
        consts = ctx.enter_context(tc.tile_pool(name="consts", bufs=1))
        xpool = ctx.enter_context(tc.tile_pool(name="xpool", bufs=2))
        sqpool = ctx.enter_context(tc.tile_pool(name="sqpool", bufs=2))
        stats = ctx.enter_context(tc.tile_pool(name="stats", bufs=2))
        opool = ctx.enter_context(tc.tile_pool(name="opool", bufs=2))
        psum_a = ctx.enter_context(tc.tile_pool(name="psum_a", bufs=2, space="PSUM"))
        psum_s = ctx.enter_context(tc.tile_pool(name="psum_s", bufs=1, space="PSUM"))
        psum_o = ctx.enter_context(tc.tile_pool(name="psum_o", bufs=4, space="PSUM"))
        psum_w = ctx.enter_context(tc.tile_pool(name="psum_w", bufs=1, space="PSUM"))

        eps_sb = consts.tile([2, 1], f32)
        nc.vector.memset(eps_sb, 1e-5)

        # PE warm-up + Sqrt ACT table pre-load (the table load otherwise sits
        # on the stats critical path mid-kernel).
        bf16 = mybir.dt.bfloat16
        wl = consts.tile([128, 128], bf16)
        nc.vector.memset(wl, 1.0)
        wr = consts.tile([128, 512], bf16)
        nc.vector.memset(wr, 1.0)
        dsq = consts.tile([2, 1], f32)
        nc.scalar.activation(dsq[:, :], eps_sb[:, :],
                             mybir.ActivationFunctionType.Sqrt,
                             bias=eps_sb[:, :])
        pw = psum_w.tile([128, 512], f32)
        for _ in range(10):
            nc.tensor.matmul(pw[:, :], lhsT=wl[:, :], rhs=wr[:, :],
                             start=True, stop=True)

        # x loads: three DMAs of two t-blocks per batch (each dma_start costs
        # ~0.9us of issue time on its sequencer; thirds let phase A start
        # after 1/3 of the input has landed). All DMAs issue on Sync.
        # Order: first third of x0, then W1 (both gate the first matmul).
        TH = NT // 3
        x_tiles = []
        x_srcs = [xs[b].rearrange("(p a) e -> p a e", a=NT)
                  for b in range(B_PER_CORE)]
        for b in range(B_PER_CORE):
            x_tiles.append(xpool.tile([TT, NT, EP], f32r, tag=f"x_{b}",
                                      name=f"x_{b}"))
        nc.sync.dma_start(out=x_tiles[0][:, 0:TH, :], in_=x_srcs[0][:, 0:TH, :])
        w1_r = consts.tile([TT, NT, M1], f32r)
        nc.sync.dma_start(out=w1_r, in_=w1[:].rearrange("(p a) m -> p a m", a=NT))
        for b in range(B_PER_CORE):
            for h in range(3):
                if b == 0 and h == 0:
                    continue
                nc.sync.dma_start(out=x_tiles[b][:, h * TH:(h + 1) * TH, :],
                                  in_=x_srcs[b][:, h * TH:(h + 1) * TH, :])
            if b == 0:
                w2_r = consts.tile([M1, NT, TT], f32r)
                nc.sync.dma_start(out=w2_r, in_=w2[:])

        # phase A for all batches
        p1s, pss = [], []
        for b in range(B_PER_CORE):
            x_r = x_tiles[b]
            xsq = sqpool.tile([TT, NT, EP], f32r, tag=f"xsq_{b}")
            p1 = psum_a.tile([M1, EP], f32)
            ps = psum_s.tile([2, EP], f32)
            p1s.append(p1)
            pss.append(ps)
            for h in range(3):
                nc.scalar.square(xsq[:, h * TH:(h + 1) * TH, :],
                                 x_r[:, h * TH:(h + 1) * TH, :])
            for ti in range(NT):
                nc.tensor.matmul(p1[:, :], lhsT=w1_r[:, ti, :],
                                 rhs=x_r[:, ti, :],
                                 start=(ti == 0), stop=(ti == NT - 1))
                nc.tensor.matmul(ps[:, :], lhsT=w1_r[:, 0, 0:2],
                                 rhs=xsq[:, ti, :],
                                 start=(ti == 0), stop=(ti == NT - 1))

        # stats for all batches (emitted before any phase-C copies so the
        # DVE runs batch 1's stats ahead of batch 0's output copies)
        rhs2s = []
        for b in range(B_PER_CORE):
            p1, ps = p1s[b], pss[b]
            # rhs2 row 0 = mu, row 1 = sd, rows 2..65 = U
            rhs2 = stats.tile([M1, EP], f32r, tag=f"rhs2_{b}")
            va = stats.tile([2, EP], f32)
            vb = stats.tile([2, EP], f32)
            vc = stats.tile([2, EP], f32)
            rhs2s.append(rhs2)
            nc.vector.tensor_copy(rhs2[:, :], p1[:, :])                  # U (+junk rows 0,1)
            nc.vector.tensor_scalar_mul(va[:, :], ps[:, :], 1.0 / T)     # E[x^2]
            nc.vector.tensor_scalar_mul(vb[:, :], p1[0:2, :], 1.0 / T)   # mu
            nc.vector.tensor_mul(vc[:, :], vb[:, :], vb[:, :])           # mu^2
            nc.vector.tensor_sub(va[:, :], va[:, :], vc[:, :])           # var
            nc.scalar.activation(rhs2[0:2, :], va[:, :],
                                 mybir.ActivationFunctionType.Sqrt,
                                 bias=eps_sb[0:2, :])                    # sd -> rows 0,1
            nc.vector.tensor_copy(rhs2[0:1, :], vb[0:1, :])              # mu -> row 0

        # phase C + stores for all batches; stores per-third on Sync (loads
        # and stores are temporally disjoint there)
        for b in range(B_PER_CORE):
            rhs2 = rhs2s[b]
            out_sb = opool.tile([TT, NT, E], f32)
            out_dst = out[b].rearrange("(p a) e -> p a e", a=NT)
            for a in range(NT):
                po = psum_o.tile([TT, EP], f32)
                nc.tensor.matmul(po[:, :], lhsT=w2_r[:, a, :],
                                 rhs=rhs2[:, :], start=True, stop=True)
                nc.vector.tensor_copy(out_sb[:, a, :], po[:, 0:E])
                if a % TH == TH - 1:
                    h = a // TH
                    nc.sync.dma_start(out=out_dst[:, h * TH:(h + 1) * TH, :],
                                      in_=out_sb[:, h * TH:(h + 1) * TH, :])

    nc.compile()
    return nc


def _get_program():
    global _PROGRAM
    if _PROGRAM is None:
        _PROGRAM = _build_program()
    return _PROGRAM


def _prepare_inputs(x_enc, A, B_vec, eval_matrix, W_mlp, b_mlp):
    x = np.asarray(x_enc, np.float32)
    xp = np.zeros((BATCH, T, EP), np.float32)
    xp[:, :, :E] = x
    W1, W2 = _fold_weights(A, B_vec, eval_matrix, W_mlp, b_mlp)
    return [
        {
            "xs": np.ascontiguousarray(xp[k * B_PER_CORE:(k + 1) * B_PER_CORE]),
            "w1": W1,
            "w2": W2,
        }
        for k in range(N_CORES)
    ]


def kernel(x_enc, A, B_vec, eval_matrix, W_mlp, b_mlp, affine_weight, affine_bias):
    from concourse.bass_utils import run_bass_kernel_spmd

    nc = _get_program()
    in_maps = _prepare_inputs(x_enc, A, B_vec, eval_matrix, W_mlp, b_mlp)
    res = run_bass_kernel_spmd(nc, in_maps, core_ids=list(range(N_CORES)))
    return np.concatenate([res.results[k]["out"] for k in range(N_CORES)], axis=0)



# revision 9
# speedup vs baseline: 1.0948x; 1.0948x over previous
"""Trainium2 Bass kernel for nn_Decomp_Forecast (HiPPO-LegS decomposition forecaster).

Math: the reference runs a 720-step linear scan c_t = c_{t-1} @ A^T + f_t * B
and only uses the final state, so the whole model collapses (exactly, by
associativity) to two chained matmuls around the instance-norm statistics:

    G[t]   = B^T (A^T)^(T-1-t)            (host-folded, float64)  [720, 64]
    P      = eval_matrix @ W_mlp                                   [720, 64]
    v      = eval_matrix @ b_mlp                                   [720]
    q      = P @ sum_t G[t]                                        [720]

    U      = x_row @ G      (x_row = raw x_enc[b, :, e], no normalization!)
    mu     = mean_t(x_row);  sd = sqrt(var_t(x_row) + 1e-5)
    out[t', r] = (P @ U)[t'] + mu_r * (1 - q[t']) + sd_r * v[t']

(the affine weight/bias are ones/zeros per the model setup, and the RevIN
scale cancels through the linear path, leaving the rank-2 mu/sd correction,
which is folded into the second matmul as two extra contraction rows.)

This version runs everything in fp16 (tolerance is 2e-2; fp16 keeps ~5e-4):
  - halves HBM traffic in both directions (host casts x to fp16, output
    returned as fp16 and upcast on host)
  - fp16 matmuls stream 1 moving-row/cycle on the PE (fp32r was ~2x slower
    in practice at 322 columns)
  - mu/sd derived without any extra scaling passes: psum already holds
    sum(x) and sum(x^2); rhs2 row0 = sum(x) with W2 row0 = (1-q)/T, and
    rhs2 row1 = sqrt(T*sum(x^2) - sum(x)^2 + T^2*eps) = T*sd with
    W2 row1 = v/T.

Device kernel per core (2 batches of the 16, data-parallel over batch):
  - time dim mapped as t = p*6 + a (p = SBUF partition, a = column block) so
    every DMA moves contiguous >=644B runs per partition (descriptor-eff.)
  - DMA issues spread over the three descriptor-gen paths (SP + Activation
    share HWDGE, Pool has its own SWDGE) to unserialize the ~0.6-1us DGE
    cost per dma_start; x/out are split in halves for latency
  - a short train of warmup matmuls runs while the first x DMA is in
    flight, so the PE HAM activity window flips to the 2.4GHz clock early
  phase A: per batch, 6 matmuls [120t x 66] x [120t x 322e] -> psum [66, 322]
           (rows 0,1 = sum_t x via two ones cols, rows 2..65 = U^T) and
           6 matmuls of the squared tiles with the 2 ones cols -> sum_t x^2
  phase B: tiny [1, 322] vector ops -> rhs2 row0 = sum x, row1 = T*sd
  phase C: per batch, 6 matmuls [66 x 120] x [66 x 322] -> psum -> fp16
           copies (spread over Pool/DVE/Act) -> 2 store DMAs per batch
"""

import numpy as np

BATCH, T, E, N = 16, 720, 321, 64
N_CORES = 8
B_PER_CORE = BATCH // N_CORES   # 2
TT = 120                        # time-tile (partition dim of phase-A matmuls)
NT = 6                          # column blocks per partition (t = p*6 + a)
M1 = N + 2                      # 66: two ones columns + G columns
EP = E + 1                      # 322: keep the moving dim even / 4B-aligned
NWARM = 7                       # PE warmup matmuls (fill the first DMA wait)

_PROGRAM = None


def _fold_weights(A, B_vec, eval_matrix, W_mlp, b_mlp):
    """Host-side weight folding in float64.

    Returns W1 [120, 6, 66] fp16 (cols: [1, 1, G], t = p*6 + a) and
    W2 [66, 6, 120] fp16 (rows: [v/T, (1-q)/T, P^T], t' = p*6 + a).
    """
    A64 = np.asarray(A, np.float64)
    Bv = np.asarray(B_vec, np.float64)
    G = np.empty((T, N), np.float64)
    r = Bv.copy()                       # r_k = B^T (A^T)^k
    for k in range(T):
        G[T - 1 - k] = r
        r = r @ A64.T
    P_mat = np.asarray(eval_matrix, np.float64) @ np.asarray(W_mlp, np.float64)
    v = np.asarray(eval_matrix, np.float64) @ np.asarray(b_mlp, np.float64)
    q = P_mat @ G.sum(axis=0)
    W1 = np.concatenate([np.ones((T, 2)), G], axis=1)           # [720, 66]
    W1 = W1.reshape(TT, NT, M1).astype(np.float16)              # t = p*6+a
    W2 = np.concatenate(
        [(v / T)[None, :], ((1.0 - q) / T)[None, :], P_mat.T], axis=0
    )                                                            # [66, 720]
    W2 = W2.reshape(M1, TT, NT).transpose(0, 2, 1).astype(np.float16)
    return np.ascontiguousarray(W1), np.ascontiguousarray(W2)


def _build_program():
    from contextlib import ExitStack

    import concourse.tile as tile
    from concourse import bacc, mybir

    f32 = mybir.dt.float32
    f16 = mybir.dt.float16
    nc = bacc.Bacc("TRN2", target_bir_lowering=False, debug=False,
                   num_devices=N_CORES)

    xs = nc.dram_tensor("xs", [B_PER_CORE, T, EP], f16, kind="ExternalInput")
    w1 = nc.dram_tensor("w1", [TT, NT, M1], f16, kind="ExternalInput")
    w2 = nc.dram_tensor("w2", [M1, NT, TT], f16, kind="ExternalInput")
    out = nc.dram_tensor("out", [B_PER_CORE, T, E], f16, kind="ExternalOutput")

    with tile.TileContext(nc) as tc, ExitStack() as ctx:
        consts = ctx.enter_context(tc.tile_pool(name="consts", bufs=1))
        xpool = ctx.enter_context(tc.tile_pool(name="xpool", bufs=1))
        sqpool = ctx.enter_context(tc.tile_pool(name="sqpool", bufs=1))
        stats = ctx.enter_context(tc.tile_pool(name="stats", bufs=1))
        opool = ctx.enter_context(tc.tile_pool(name="opool", bufs=1))
        psum_a = ctx.enter_context(tc.tile_pool(name="psum_a", bufs=1, space="PSUM"))
        psum_s = ctx.enter_context(tc.tile_pool(name="psum_s", bufs=1, space="PSUM"))
        psum_o = ctx.enter_context(tc.tile_pool(name="psum_o", bufs=4, space="PSUM"))

        # ---- constants / warmup sources (no DMA deps) ----
        eps2 = consts.tile([2, 1], f32)
        nc.vector.memset(eps2, float(T) * float(T) * 1e-5)
        wl = consts.tile([128, 64], f16)
        nc.vector.memset(wl, 1.0)
        wr = consts.tile([128, 256], f16)
        nc.vector.memset(wr, 1.0)
        # Sqrt ACT table pre-load (otherwise it lands mid-kernel on the
        # stats critical path).
        dsq = consts.tile([2, 1], f32)
        nc.scalar.square(dsq[:, :], eps2[:, :])
        nc.scalar.activation(dsq[:, :], eps2[:, :],
                             mybir.ActivationFunctionType.Sqrt,
                             bias=eps2[:, :])

        # ---- input DMAs, spread across the three DGE paths ----
        HH = NT // 2                                   # half = 3 a-blocks
        x_tiles = [xpool.tile([TT, NT, EP], f16, tag=f"x_{b}", name=f"x_{b}")
                   for b in range(B_PER_CORE)]
        x_srcs = [xs[b].rearrange("(p a) e -> p a e", a=NT)
                  for b in range(B_PER_CORE)]
        w1_r = consts.tile([TT, NT, M1], f16)
        nc.sync.dma_start(out=w1_r, in_=w1[:])
        nc.sync.dma_start(out=x_tiles[0][:, 0:HH, :], in_=x_srcs[0][:, 0:HH, :])
        nc.sync.dma_start(out=x_tiles[0][:, HH:NT, :], in_=x_srcs[0][:, HH:NT, :])
        nc.gpsimd.dma_start(out=x_tiles[1][:, 0:HH, :], in_=x_srcs[1][:, 0:HH, :])
        nc.gpsimd.dma_start(out=x_tiles[1][:, HH:NT, :], in_=x_srcs[1][:, HH:NT, :])
        w2_r = consts.tile([M1, NT, TT], f16)
        nc.sync.dma_start(out=w2_r, in_=w2[:])

        # ---- PE warmup while x streams in ----
        pw = psum_o.tile([64, 256], f32, tag="po")
        for _ in range(NWARM):
            nc.tensor.matmul(pw[:, :], lhsT=wl[:, :64], rhs=wr[:, :],
                             start=True, stop=True)

        # ---- squares (ScalarE for b0 in halves chasing the DMAs; Pool b1) ----
        xsq = [sqpool.tile([TT, NT, EP], f16, tag=f"xsq_{b}", name=f"xsq_{b}")
               for b in range(B_PER_CORE)]
        nc.scalar.square(xsq[0][:, 0:HH, :], x_tiles[0][:, 0:HH, :])
        nc.scalar.square(xsq[0][:, HH:NT, :], x_tiles[0][:, HH:NT, :])
        nc.vector.tensor_mul(xsq[1][:, 0:HH, :], x_tiles[1][:, 0:HH, :],
                             x_tiles[1][:, 0:HH, :])
        nc.vector.tensor_mul(xsq[1][:, HH:NT, :], x_tiles[1][:, HH:NT, :],
                             x_tiles[1][:, HH:NT, :])

        # ---- phase A ----
        p1s, pss = [], []
        for b in range(B_PER_CORE):
            p1 = psum_a.tile([M1, EP], f32, name=f"p1_{b}")
            ps = psum_s.tile([2, EP], f32, name=f"ps_{b}")
            p1s.append(p1)
            pss.append(ps)
        for b in range(B_PER_CORE):
            for ti in range(NT):
                nc.tensor.matmul(p1s[b][:, :], lhsT=w1_r[:, ti, :],
                                 rhs=x_tiles[b][:, ti, :],
                                 start=(ti == 0), stop=(ti == NT - 1))
            for ti in range(NT):
                nc.tensor.matmul(pss[b][:, :], lhsT=w1_r[:, ti, 0:2],
                                 rhs=xsq[b][:, ti, :],
                                 start=(ti == 0), stop=(ti == NT - 1))

        # ---- stats (phase B) ----
        # rhs2 row0 = T*sd (-> W2 row0 v/T), row1 = sum x (-> W2 row1 (1-q)/T)
        rhs2s, vvs = [], []
        for b in range(B_PER_CORE):
            rhs2 = stats.tile([M1, EP], f16, tag=f"rhs2_{b}", name=f"rhs2_{b}")
            vc = stats.tile([2, EP], f32, tag=f"vc_{b}", name=f"vc_{b}")
            vv = stats.tile([2, EP], f32, tag=f"vv_{b}", name=f"vv_{b}")
            rhs2s.append(rhs2)
            vvs.append(vv)
            nc.vector.tensor_copy(rhs2[:, :], p1s[b][:, :])     # casts to fp16
            nc.vector.tensor_mul(vc[:, :], rhs2[0:2, :], rhs2[0:2, :])
            nc.vector.scalar_tensor_tensor(
                vv[:, :], pss[b][0:2, :], float(T), vc[:, :],
                op0=mybir.AluOpType.mult, op1=mybir.AluOpType.subtract)
        nc.scalar.activation(rhs2s[0][0:1, :], vvs[0][0:1, :],
                             mybir.ActivationFunctionType.Sqrt,
                             bias=eps2[0:1, :])

        # ---- phase C + copies + stores ----
        copy_engines = [nc.vector, nc.scalar]

        def phase_c(b):
            out_sb = opool.tile([TT, NT, E], f16, tag=f"out_{b}", name=f"out_{b}")
            out_dst = out[b].rearrange("(p a) e -> p a e", a=NT)
            for a in range(NT):
                po = psum_o.tile([TT, EP], f32, tag="po", name=f"po_{b}_{a}")
                nc.tensor.matmul(po[:, :], lhsT=w2_r[:, a, :],
                                 rhs=rhs2s[b][:, :], start=True, stop=True)
                eng = copy_engines[a % 2]
                if eng is nc.scalar:
                    eng.copy(out_sb[:, a, :], po[:, 0:E])
                else:
                    eng.tensor_copy(out_sb[:, a, :], po[:, 0:E])
                if a == HH - 1:
                    pass
            # store halves on separate DGE paths
            eng_st = nc.sync if b == 0 else nc.gpsimd
            eng_st.dma_start(out=out_dst[:, 0:HH, :], in_=out_sb[:, 0:HH, :])
            eng_st.dma_start(out=out_dst[:, HH:NT, :], in_=out_sb[:, HH:NT, :])

        phase_c(0)
        nc.scalar.activation(rhs2s[1][0:1, :], vvs[1][0:1, :],
                             mybir.ActivationFunctionType.Sqrt,
                             bias=eps2[0:1, :])
        phase_c(1)

    nc.compile()
    return nc


def _get_program():
    global _PROGRAM
    if _PROGRAM is None:
        _PROGRAM = _build_program()
    return _PROGRAM


def _prepare_inputs(x_enc, A, B_vec, eval_matrix, W_mlp, b_mlp):
    x = np.asarray(x_enc, np.float32)
    xp = np.zeros((BATCH, T, EP), np.float16)
    xp[:, :, :E] = x
    W1, W2 = _fold_weights(A, B_vec, eval_matrix, W_mlp, b_mlp)
    return [
        {
            "xs": np.ascontiguousarray(xp[k * B_PER_CORE:(k + 1) * B_PER_CORE]),
            "w1": W1,
            "w2": W2,
        }
        for k in range(N_CORES)
    ]


def kernel(x_enc, A, B_vec, eval_matrix, W_mlp, b_mlp, affine_weight, affine_bias):
    from concourse.bass_utils import run_bass_kernel_spmd

    nc = _get_program()
    in_maps = _prepare_inputs(x_enc, A, B_vec, eval_matrix, W_mlp, b_mlp)
    res = run_bass_kernel_spmd(nc, in_maps, core_ids=list(range(N_CORES)))
    return np.concatenate(
        [res.results[k]["out"] for k in range(N_CORES)], axis=0
    ).astype(np.float32)


# revision 11
# speedup vs baseline: 1.1918x; 1.0886x over previous
"""Trainium2 Bass kernel for nn_Decomp_Forecast (HiPPO-LegS decomposition forecaster).

Math: the reference runs a 720-step linear scan c_t = c_{t-1} @ A^T + f_t * B
and only uses the final state, so the whole model collapses (exactly, by
associativity) to two chained matmuls around the instance-norm statistics:

    G[t]   = B^T (A^T)^(T-1-t)            (host-folded, float64)  [720, 64]
    P      = eval_matrix @ W_mlp                                   [720, 64]
    v      = eval_matrix @ b_mlp                                   [720]
    q      = P @ sum_t G[t]                                        [720]

    U      = x_row @ G      (x_row = raw x_enc[b, :, e], no normalization!)
    mu     = mean_t(x_row);  sd = sqrt(var_t(x_row) + 1e-5)
    out[t', r] = (P @ U)[t'] + mu_r * (1 - q[t']) + sd_r * v[t']

(the affine weight/bias are ones/zeros per the model setup, and the RevIN
scale cancels through the linear path, leaving the rank-2 mu/sd correction,
which is folded into the second matmul as two extra contraction rows.)

Everything runs in fp16 (tolerance 2e-2, this kernel lands ~7e-4): halves
HBM traffic both ways and fp16 matmuls stream 1 moving-row/cycle on the PE.

Per-core device schedule (2 of the 16 batches per core):
  - t = p*6 + a (p = SBUF partition, a = column block) so every DMA half
    moves 1932B contiguous runs per partition
  - x loads ride the Pool-engine SWDGE path (it coalesces descriptors and
    its DGE is independent of the SP/Act HWDGE), in halves, strictly
    ordered b0h0, b0h1, b1h0, b1h1 so batch 0 lands first; weights ride
    the SP HWDGE path concurrently
  - a short warmup matmul train runs during the initial DMA wait so the
    PE HAM activity window flips to the 2.4GHz clock before phase A
  - phase A per batch: 6 matmuls lhsT=[G|1|1] -> psum rows 0:64 = U^T,
    rows 64:66 = sum(x); 6 matmuls of the squared tiles against the ones
    pair -> sum(x^2), both batches sharing ONE psum bank (rows 0:2 and
    32:34, 32-aligned), saving a psum bank for the output pool
  - stats: (T*sum(x^2) - sum(x)^2) via 2 DVE ops, sqrt on ScalarE ->
    rhs2 = [U (0:64), T*sd (64), sum(x) (65)] fp16
  - phase C per batch: 6 matmuls lhsT=W2[:, a-block] [66, 120], rhs=rhs2
    -> psum [120, 322] -> fp16 copies (DVE/Act alternating) -> stores in
    halves (batch 0 via SP, batch 1 via SWDGE)
"""

import numpy as np

BATCH, T, E, N = 16, 720, 321, 64
N_CORES = 8
B_PER_CORE = BATCH // N_CORES   # 2
TT = 120                        # time-tile (partition dim of phase-A matmuls)
NT = 6                          # column blocks per partition (t = p*6 + a)
M1 = N + 2                      # 66: G columns + two ones columns
W1C = 66                        # W1 stationary cols: [G(64), 1, 1]
M2 = M1                         # phase-C contraction rows: [P^T, v/T, (1-q)/T]
EP = E + 1                      # 322: keep the moving dim even / 4B-aligned
NWARM = 6                       # PE warmup matmuls (fill the first DMA wait)

_PROGRAM = None


def _fold_weights(A, B_vec, eval_matrix, W_mlp, b_mlp):
    """Host-side weight folding in float64.

    W1 [120, 6, 66] fp16: cols 0:64 = G (t = p*6 + a), cols 64,65 = 1
    (-> psum rows 64:66 = sum x; also used against the squared tiles for
    sum x^2).
    W2 [66, 6, 120] fp16: rows [P^T (0:64), v/T (64), (1-q)/T (65)],
    columns regrouped so block a holds t' = p*6 + a.
    """
    A64 = np.asarray(A, np.float64)
    Bv = np.asarray(B_vec, np.float64)
    G = np.empty((T, N), np.float64)
    r = Bv.copy()                       # r_k = B^T (A^T)^k
    for k in range(T):
        G[T - 1 - k] = r
        r = r @ A64.T
    P_mat = np.asarray(eval_matrix, np.float64) @ np.asarray(W_mlp, np.float64)
    v = np.asarray(eval_matrix, np.float64) @ np.asarray(b_mlp, np.float64)
    q = P_mat @ G.sum(axis=0)
    W1 = np.zeros((T, W1C))
    W1[:, 0:N] = G
    W1[:, N:N + 2] = 1.0
    W1 = W1.reshape(TT, NT, W1C).astype(np.float16)             # t = p*6+a
    W2 = np.concatenate(
        [P_mat.T, (v / T)[None, :], ((1.0 - q) / T)[None, :]], axis=0
    )                                                            # [66, 720]
    W2 = W2.reshape(M2, TT, NT).transpose(0, 2, 1).astype(np.float16)
    return np.ascontiguousarray(W1), np.ascontiguousarray(W2)


def _build_program():
    from contextlib import ExitStack

    import concourse.tile as tile
    from concourse import bacc, mybir

    f32 = mybir.dt.float32
    f16 = mybir.dt.float16
    nc = bacc.Bacc("TRN2", target_bir_lowering=False, debug=False,
                   num_devices=N_CORES)

    xs = nc.dram_tensor("xs", [B_PER_CORE, T, EP], f16, kind="ExternalInput")
    w1 = nc.dram_tensor("w1", [TT, NT, W1C], f16, kind="ExternalInput")
    w2 = nc.dram_tensor("w2", [M2, NT, TT], f16, kind="ExternalInput")
    out = nc.dram_tensor("out", [B_PER_CORE, T, E], f16, kind="ExternalOutput")

    with tile.TileContext(nc) as tc, ExitStack() as ctx:
        consts = ctx.enter_context(tc.tile_pool(name="consts", bufs=1))
        xpool = ctx.enter_context(tc.tile_pool(name="xpool", bufs=1))
        sqpool = ctx.enter_context(tc.tile_pool(name="sqpool", bufs=1))
        stats = ctx.enter_context(tc.tile_pool(name="stats", bufs=1))
        opool = ctx.enter_context(tc.tile_pool(name="opool", bufs=1))
        psum_a = ctx.enter_context(tc.tile_pool(name="psum_a", bufs=1, space="PSUM"))
        psum_s = ctx.enter_context(tc.tile_pool(name="psum_s", bufs=1, space="PSUM"))
        psum_o = ctx.enter_context(tc.tile_pool(name="psum_o", bufs=5, space="PSUM"))

        # ---- constants / warmup sources (no DMA deps) ----
        eps2 = consts.tile([2, 1], f32)
        nc.vector.memset(eps2, float(T) * float(T) * 1e-5)
        wl = consts.tile([128, 64], f16)
        nc.vector.memset(wl, 1.0)
        wr = consts.tile([128, 256], f16)
        nc.vector.memset(wr, 1.0)
        # ACT table pre-loads (Square first, then Sqrt: both loads land in
        # the preamble instead of on the stats critical path).
        dsq = consts.tile([2, 1], f32)
        nc.scalar.square(dsq[:, :], eps2[:, :])
        nc.scalar.activation(dsq[:, :], eps2[:, :],
                             mybir.ActivationFunctionType.Sqrt,
                             bias=eps2[:, :])

        # ---- input DMAs ----
        # x rides SWDGE (Pool) in order b0h0, b0h1, b1h0, b1h1; weights ride
        # the SP HWDGE concurrently.
        HH = NT // 2                                   # half = 3 a-blocks
        x_tiles = [xpool.tile([TT, NT, EP], f16, tag=f"x_{b}", name=f"x_{b}")
                   for b in range(B_PER_CORE)]
        x_srcs = [xs[b].rearrange("(p a) e -> p a e", a=NT)
                  for b in range(B_PER_CORE)]
        w1_r = consts.tile([TT, NT, W1C], f16)
        nc.sync.dma_start(out=w1_r, in_=w1[:])
        for b in range(B_PER_CORE):
            nc.gpsimd.dma_start(out=x_tiles[b][:, 0:HH, :],
                                in_=x_srcs[b][:, 0:HH, :])
            nc.gpsimd.dma_start(out=x_tiles[b][:, HH:NT, :],
                                in_=x_srcs[b][:, HH:NT, :])
        w2_r = consts.tile([M2, NT, TT], f16)
        nc.sync.dma_start(out=w2_r, in_=w2[:])

        # ---- PE warmup while x streams in ----
        pw = psum_o.tile([64, 256], f32, tag="po", name="pw")
        for _ in range(NWARM):
            nc.tensor.matmul(pw[:, :], lhsT=wl[:, :64], rhs=wr[:, :],
                             start=True, stop=True)

        # ---- squares chasing the DMA halves (ScalarE h0, DVE h1) ----
        xsq = [sqpool.tile([TT, NT, EP], f16, tag=f"xsq_{b}", name=f"xsq_{b}")
               for b in range(B_PER_CORE)]
        for b in range(B_PER_CORE):
            nc.scalar.square(xsq[b][:, 0:HH, :], x_tiles[b][:, 0:HH, :])
            nc.vector.tensor_mul(xsq[b][:, HH:NT, :], x_tiles[b][:, HH:NT, :],
                                 x_tiles[b][:, HH:NT, :])

        # ---- phase A ----
        p1s = [psum_a.tile([M1, EP], f32, name=f"p1_{b}")
               for b in range(B_PER_CORE)]
        ps = psum_s.tile([34, EP], f32, name="ps")
        sq_rows = [(0, 2), (32, 34)]
        for b in range(B_PER_CORE):
            for ti in range(NT):
                nc.tensor.matmul(p1s[b][0:M1, :], lhsT=w1_r[:, ti, 0:M1],
                                 rhs=x_tiles[b][:, ti, :],
                                 start=(ti == 0), stop=(ti == NT - 1))
            r0, r1 = sq_rows[b]
            for ti in range(NT):
                nc.tensor.matmul(ps[r0:r1, :], lhsT=w1_r[:, ti, 64:66],
                                 rhs=xsq[b][:, ti, :],
                                 start=(ti == 0), stop=(ti == NT - 1))

        # ---- stats (phase B) ----
        # rhs2 rows: U (0:64), T*sd (64), sum x (65)
        rhs2s, vvs = [], []
        for b in range(B_PER_CORE):
            rhs2 = stats.tile([M1, EP], f16, tag=f"rhs2_{b}", name=f"rhs2_{b}")
            vc = stats.tile([2, EP], f32, tag=f"vc_{b}", name=f"vc_{b}")
            vv = stats.tile([2, EP], f32, tag=f"vv_{b}", name=f"vv_{b}")
            rhs2s.append(rhs2)
            vvs.append(vv)
            nc.vector.tensor_copy(rhs2[0:64, :], p1s[b][0:64, :])    # U
            nc.vector.tensor_copy(rhs2[64:66, :], p1s[b][64:66, :])  # sum x (x2)
            nc.vector.tensor_mul(vc[:, :], rhs2[64:66, :], rhs2[64:66, :])
            r0, r1 = sq_rows[b]
            nc.vector.scalar_tensor_tensor(
                vv[:, :], ps[r0:r1, :], float(T), vc[:, :],
                op0=mybir.AluOpType.mult, op1=mybir.AluOpType.subtract)
        nc.scalar.activation(rhs2s[0][64:65, :], vvs[0][0:1, :],
                             mybir.ActivationFunctionType.Sqrt,
                             bias=eps2[0:1, :])

        # ---- phase C + copies + stores ----
        copy_engines = [nc.vector, nc.scalar]

        def phase_c(b):
            out_sb = opool.tile([TT, NT, E], f16, tag=f"out_{b}",
                                name=f"out_{b}")
            out_dst = out[b].rearrange("(p a) e -> p a e", a=NT)
            for a in range(NT):
                po = psum_o.tile([TT, EP], f32, tag="po", name=f"po_{b}_{a}")
                nc.tensor.matmul(po[:, :], lhsT=w2_r[:, a, :],
                                 rhs=rhs2s[b][:, :], start=True, stop=True)
                eng = copy_engines[a % 2]
                if eng is nc.scalar:
                    eng.copy(out_sb[:, a, :], po[:, 0:E])
                else:
                    eng.tensor_copy(out_sb[:, a, :], po[:, 0:E])
            eng_st = nc.sync if b == 0 else nc.gpsimd
            eng_st.dma_start(out=out_dst[:, 0:HH, :], in_=out_sb[:, 0:HH, :])
            eng_st.dma_start(out=out_dst[:, HH:NT, :], in_=out_sb[:, HH:NT, :])

        phase_c(0)
        nc.scalar.activation(rhs2s[1][64:65, :], vvs[1][0:1, :],
                             mybir.ActivationFunctionType.Sqrt,
                             bias=eps2[0:1, :])
        phase_c(1)

    nc.compile()
    return nc


def _get_program():
    global _PROGRAM
    if _PROGRAM is None:
        _PROGRAM = _build_program()
    return _PROGRAM


def _prepare_inputs(x_enc, A, B_vec, eval_matrix, W_mlp, b_mlp):
    x = np.asarray(x_enc, np.float32)
    xp = np.zeros((BATCH, T, EP), np.float16)
    xp[:, :, :E] = x
    W1, W2 = _fold_weights(A, B_vec, eval_matrix, W_mlp, b_mlp)
    return [
        {
            "xs": np.ascontiguousarray(xp[k * B_PER_CORE:(k + 1) * B_PER_CORE]),
            "w1": W1,
            "w2": W2,
        }
        for k in range(N_CORES)
    ]


def kernel(x_enc, A, B_vec, eval_matrix, W_mlp, b_mlp, affine_weight, affine_bias):
    from concourse.bass_utils import run_bass_kernel_spmd

    nc = _get_program()
    in_maps = _prepare_inputs(x_enc, A, B_vec, eval_matrix, W_mlp, b_mlp)
    res = run_bass_kernel_spmd(nc, in_maps, core_ids=list(range(N_CORES)))
    return np.concatenate(
        [res.results[k]["out"] for k in range(N_CORES)], axis=0
    ).astype(np.float32)
